# revision 10
# baseline (speedup 1.0000x reference)
# GQA multi-head attention (B=2, S=2048, E=4096, 32 q-heads / 8 kv-heads,
# head_dim=128, NeoX RoPE, causal) for 8 trn2 NeuronCores.
#
# Sharding: 2 (batch) x 4 (kv-head-group) = 8 cores. Core c = 4*b + g handles
# batch b with kv-heads {2g, 2g+1} / q-heads 8g..8g+7. Each core emits a
# partial dense output [S, E]; the host sums the 4 group-partials per batch.
#
# All matmuls run in bf16 (fp32 PSUM accumulation). Host prepares:
#   - per-batch transposed+tiled activations (so the kernel needs no on-chip
#     transposes of x, and every DMA is contiguous)
#   - per-group transposed weight slices in bf16
#   - RoPE cos/sin half tables, causal mask tile, identity tile
import numpy as np
import ml_dtypes

import concourse.bass as bass
import concourse.bacc as bacc
import concourse.mybir as mybir
import concourse.tile as tile
from concourse.bass_utils import run_bass_kernel_spmd

BF16 = ml_dtypes.bfloat16
F32 = np.float32

B = 2
S = 2048
E = 4096
NHEADS = 32
KVHEADS = 8
D = 128  # head dim
P = 128  # partitions
ROPE_BASE = 10000.0

NCORES = 8
NB = 2          # batch shards
NG = 4          # head-group shards
HQ = NHEADS // NG      # 8 q-heads per core
HKV = KVHEADS // NG    # 2 kv-heads per core
NQ = HQ * D            # 1024 q-proj cols per core
NKV = HKV * D          # 256 k/v-proj cols per core

ST = S // P    # 16 s-tiles
EC = E // P    # 32 e-chunks (contraction)
CC = NQ // P   # 8 ctx chunks (dense contraction)
OG = E // 512  # 8 output column groups of 512

INV_SQRT_D = 1.0 / float(np.sqrt(D))
MASK_NEG = -1.0e9

fp32 = mybir.dt.float32
bf16 = mybir.dt.bfloat16


def _build_module() -> bass.Bass:
    nc = bacc.Bacc()

    # --- DRAM parameters (per-core views; host supplies the right slices) ---
    xtq = nc.declare_dram_parameter("xtq", [ST, P, EC, P], bf16, isOutput=False)
    xtk = nc.declare_dram_parameter("xtk", [ST, P, EC, P], bf16, isOutput=False)
    xtv = nc.declare_dram_parameter("xtv", [ST, P, EC, P], bf16, isOutput=False)
    wqt = nc.declare_dram_parameter("wqt", [P, EC, NQ], bf16, isOutput=False)
    wkvt = nc.declare_dram_parameter("wkvt", [P, EC, 2 * NKV], bf16, isOutput=False)
    wdt = nc.declare_dram_parameter("wdt", [P, CC, E], bf16, isOutput=False)
    cosh = nc.declare_dram_parameter("cosh", [P, ST, 64], fp32, isOutput=False)
    sinh = nc.declare_dram_parameter("sinh", [P, ST, 64], fp32, isOutput=False)
    maskt = nc.declare_dram_parameter("maskt", [P, P], fp32, isOutput=False)
    ident = nc.declare_dram_parameter("ident", [P, P], bf16, isOutput=False)
    out = nc.declare_dram_parameter("out", [S, E], fp32, isOutput=True)

    with tile.TileContext(nc) as tc:
        from contextlib import ExitStack

        with ExitStack() as ctx:
            # --- pools ---
            const = ctx.enter_context(tc.tile_pool(name="const", bufs=1))
            bigw = ctx.enter_context(tc.tile_pool(name="bigw", bufs=1))
            res = ctx.enter_context(tc.tile_pool(name="res", bufs=1))
            xt_pool = ctx.enter_context(tc.tile_pool(name="xt", bufs=2))
            nat_pool = ctx.enter_context(tc.tile_pool(name="nat", bufs=2))
            rtmp_pool = ctx.enter_context(tc.tile_pool(name="rtmp", bufs=2))
            rq_pool = ctx.enter_context(tc.tile_pool(name="rq", bufs=2))
            attn_pool = ctx.enter_context(tc.tile_pool(name="attn", bufs=4))
            att_t_pool = ctx.enter_context(tc.tile_pool(name="attT", bufs=4))
            small_pool = ctx.enter_context(tc.tile_pool(name="small", bufs=2))
            outst_pool = ctx.enter_context(tc.tile_pool(name="outst", bufs=2))

            pp_proj = ctx.enter_context(tc.tile_pool(name="pp_proj", bufs=2, space="PSUM"))
            pp_sc = ctx.enter_context(tc.tile_pool(name="pp_sc", bufs=2, space="PSUM"))
            pp_tr = ctx.enter_context(tc.tile_pool(name="pp_tr", bufs=2, space="PSUM"))
            pp_ctx = ctx.enter_context(tc.tile_pool(name="pp_ctx", bufs=2, space="PSUM"))

            # --- constants ---
            cos_sb = const.tile([P, ST, 64], fp32, tag="cos")
            nc.gpsimd.dma_start(out=cos_sb, in_=cosh[:])
            sin_sb = const.tile([P, ST, 64], fp32, tag="sin")
            nc.gpsimd.dma_start(out=sin_sb, in_=sinh[:])
            mask_sb = const.tile([P, P], fp32, tag="mask")
            nc.gpsimd.dma_start(out=mask_sb, in_=maskt[:])
            id_sb = const.tile([P, P], bf16, tag="ident")
            nc.gpsimd.dma_start(out=id_sb, in_=ident[:])

            # --- persistent results of phase 1 ---
            qt_sb = res.tile([P, HQ, S], bf16, tag="qt")      # Q^T per head [d, s]
            kt_sb = res.tile([P, HKV, S], bf16, tag="kt")     # K^T per kv head
            vv_sb = res.tile([P, ST, HKV, D], bf16, tag="vv")  # V natural [sk, d]
            ctxT_sb = res.tile([P, CC, S], bf16, tag="ctxT")  # ctx^T for dense

            def rope_half(dst_bf16, src_f32, nh, i):
                """NeoX rotate-half RoPE. src/dst: [P, nh*128] views, s-tile i."""
                sv = src_f32.rearrange("p (h d) -> p h d", d=D)
                dv = dst_bf16.rearrange("p (h d) -> p h d", d=D)
                cos_i = cos_sb[:, i, :].unsqueeze(1).broadcast_to((P, nh, 64))
                sin_i = sin_sb[:, i, :].unsqueeze(1).broadcast_to((P, nh, 64))
                m1f = rtmp_pool.tile([P, HQ // 2, 64], fp32, tag="m1")
                m2f = rtmp_pool.tile([P, HQ // 2, 64], fp32, tag="m2")
                m1 = m1f[:, :nh, :]
                m2 = m2f[:, :nh, :]
                # out0 = q0*cos - q1*sin ; out1 = q1*cos + q0*sin
                nc.vector.tensor_mul(m1, sv[:, :, 0:64], cos_i)
                nc.vector.tensor_mul(m2, sv[:, :, 64:128], sin_i)
                nc.vector.tensor_sub(dv[:, :, 0:64], m1, m2)
                nc.vector.tensor_mul(m1, sv[:, :, 64:128], cos_i)
                nc.vector.tensor_mul(m2, sv[:, :, 0:64], sin_i)
                nc.vector.tensor_add(dv[:, :, 64:128], m1, m2)

            # =========================================================
            # Phase 1a: K/V projections + K rope + K^T + V for all s-tiles
            # =========================================================
            wkv_sb = bigw.tile([P, EC, 2 * NKV], bf16, tag="bigw")
            nc.gpsimd.dma_start(out=wkv_sb, in_=wkvt[:])

            for i in range(ST):
                xk = xt_pool.tile([P, EC, P], bf16, tag="xt")
                nc.gpsimd.dma_start(out=xk, in_=xtk[i])
                xv = xt_pool.tile([P, EC, P], bf16, tag="xt")
                nc.gpsimd.dma_start(out=xv, in_=xtv[i])

                kv_ps = pp_proj.tile([P, 512], fp32, tag="proj")
                for c in range(EC):
                    nc.tensor.matmul(kv_ps[:, 0:NKV], xk[:, c, :], wkv_sb[:, c, 0:NKV],
                                     start=(c == 0), stop=(c == EC - 1))
                for c in range(EC):
                    nc.tensor.matmul(kv_ps[:, NKV:2 * NKV], xv[:, c, :],
                                     wkv_sb[:, c, NKV:2 * NKV],
                                     start=(c == 0), stop=(c == EC - 1))
                # V: cast straight to resident natural layout
                nc.vector.tensor_copy(
                    out=vv_sb[:, i, :, :].rearrange("p h d -> p (h d)"),
                    in_=kv_ps[:, NKV:2 * NKV])
                # K: copy to fp32, rope, cast, transpose per head
                knat = nat_pool.tile([P, NKV], fp32, tag="knat")
                nc.vector.tensor_copy(out=knat, in_=kv_ps[:, 0:NKV])
                kr = rq_pool.tile([P, NKV], bf16, tag="kr")
                rope_half(kr, knat, HKV, i)
                for h in range(HKV):
                    tps = pp_tr.tile([P, P], bf16, tag="tr")
                    nc.tensor.transpose(tps, kr[:, h * D:(h + 1) * D], id_sb)
                    nc.vector.tensor_copy(out=kt_sb[:, h, i * P:(i + 1) * P], in_=tps)

            # =========================================================
            # Phase 1b: Q projection + rope + Q^T for all s-tiles
            # =========================================================
            wq_sb = bigw.tile([P, EC, NQ], bf16, tag="bigw")
            nc.gpsimd.dma_start(out=wq_sb, in_=wqt[:])

            for i in range(ST):
                xq = xt_pool.tile([P, EC, P], bf16, tag="xt")
                nc.gpsimd.dma_start(out=xq, in_=xtq[i])
                for half in range(2):
                    q_ps = pp_proj.tile([P, 512], fp32, tag="proj")
                    for c in range(EC):
                        nc.tensor.matmul(q_ps, xq[:, c, :],
                                         wq_sb[:, c, half * 512:(half + 1) * 512],
                                         start=(c == 0), stop=(c == EC - 1))
                    qnat = nat_pool.tile([P, 512], fp32, tag="qnat")
                    nc.vector.tensor_copy(out=qnat, in_=q_ps)
                    qr = rq_pool.tile([P, 512], bf16, tag="qr")
                    rope_half(qr, qnat, 4, i)
                    for hh in range(4):
                        h = half * 4 + hh
                        tps = pp_tr.tile([P, P], bf16, tag="tr")
                        nc.tensor.transpose(tps, qr[:, hh * D:(hh + 1) * D], id_sb)
                        nc.vector.tensor_copy(out=qt_sb[:, h, i * P:(i + 1) * P], in_=tps)

            # =========================================================
            # Phase 2: SDPA per q-head (causal, softmax w/o max-subtract)
            # =========================================================
            for h in range(HQ):
                g = h // (HQ // HKV)  # local kv head
                for i in range(ST):
                    nfull = i // 4          # full 512-wide sk groups
                    tailw = (i % 4 + 1) * P  # tail width incl. diagonal chunk
                    ngroups = nfull + 1
                    qt_i = qt_sb[:, h, i * P:(i + 1) * P]

                    dparts = small_pool.tile([P, 8], fp32, tag="dparts")
                    attn_tiles = []
                    for j in range(nfull):
                        sc_ps = pp_sc.tile([P, 512], fp32, tag="sc")
                        nc.tensor.matmul(sc_ps, qt_i, kt_sb[:, g, j * 512:(j + 1) * 512],
                                         start=True, stop=True)
                        a_sb = attn_pool.tile([P, 512], bf16, tag="attn")
                        nc.scalar.activation(out=a_sb, in_=sc_ps,
                                             func=mybir.ActivationFunctionType.Exp,
                                             scale=INV_SQRT_D,
                                             accum_out=dparts[:, j:j + 1])
                        attn_tiles.append((a_sb, 512))
                    # tail group: (i%4) full 128-chunks + masked diagonal chunk
                    sc_ps = pp_sc.tile([P, 512], fp32, tag="sc")
                    nc.tensor.matmul(sc_ps[:, 0:tailw], qt_i,
                                     kt_sb[:, g, nfull * 512:nfull * 512 + tailw],
                                     start=True, stop=True)
                    nc.vector.tensor_add(sc_ps[:, tailw - P:tailw],
                                         sc_ps[:, tailw - P:tailw], mask_sb)
                    a_sb = attn_pool.tile([P, 512], bf16, tag="attn")
                    nc.scalar.activation(out=a_sb[:, 0:tailw], in_=sc_ps[:, 0:tailw],
                                         func=mybir.ActivationFunctionType.Exp,
                                         scale=INV_SQRT_D,
                                         accum_out=dparts[:, nfull:nfull + 1])
                    attn_tiles.append((a_sb, tailw))

                    denom = small_pool.tile([P, 1], fp32, tag="denom")
                    nc.vector.reduce_sum(out=denom, in_=dparts[:, 0:ngroups],
                                         axis=mybir.AxisListType.X)
                    recip = small_pool.tile([P, 1], fp32, tag="recip")
                    nc.vector.reciprocal(out=recip, in_=denom)

                    # ctx accumulation over sk chunks of 128
                    ctx_ps = pp_ctx.tile([P, D], fp32, tag="ctx")
                    nchunks = i + 1
                    for c in range(nchunks):
                        a_sb_c, _w = attn_tiles[c // 4]
                        a_slice = a_sb_c[:, (c % 4) * P:(c % 4 + 1) * P]
                        at_ps = pp_tr.tile([P, P], bf16, tag="tr")
                        nc.tensor.transpose(at_ps, a_slice, id_sb)
                        at_sb = att_t_pool.tile([P, P], bf16, tag="attT")
                        nc.vector.tensor_copy(out=at_sb, in_=at_ps)
                        nc.tensor.matmul(ctx_ps, at_sb, vv_sb[:, c, g, :],
                                         start=(c == 0), stop=(c == nchunks - 1))
                    # normalize + cast
                    ctxn = small_pool.tile([P, D], bf16, tag="ctxn")
                    nc.vector.tensor_scalar_mul(out=ctxn, in0=ctx_ps, scalar1=recip)
                    # transpose into resident ctx^T (chunk index == head)
                    tps = pp_tr.tile([P, P], bf16, tag="tr")
                    nc.tensor.transpose(tps, ctxn, id_sb)
                    nc.vector.tensor_copy(out=ctxT_sb[:, h, i * P:(i + 1) * P], in_=tps)

            # =========================================================
            # Phase 3: dense partial out = ctx @ Wd_slice^T
            # =========================================================
            wd_sb = bigw.tile([P, CC, E], bf16, tag="bigw")
            nc.gpsimd.dma_start(out=wd_sb, in_=wdt[:])

            for i in range(ST):
                for og in range(OG):
                    d_ps = pp_proj.tile([P, 512], fp32, tag="proj")
                    for c in range(CC):
                        nc.tensor.matmul(d_ps, ctxT_sb[:, c, i * P:(i + 1) * P],
                                         wd_sb[:, c, og * 512:(og + 1) * 512],
                                         start=(c == 0), stop=(c == CC - 1))
                    o_sb = outst_pool.tile([P, 512], fp32, tag="outst")
                    nc.vector.tensor_copy(out=o_sb, in_=d_ps)
                    nc.gpsimd.dma_start(out=out[i * P:(i + 1) * P, og * 512:(og + 1) * 512],
                                      in_=o_sb)

    return nc


_NC_CACHE = None


def _get_module():
    global _NC_CACHE
    if _NC_CACHE is None:
        _NC_CACHE = _build_module()
        if not _NC_CACHE.is_finalized():
            _NC_CACHE.finalize()
    return _NC_CACHE


def _pack_xt(xb: np.ndarray) -> np.ndarray:
    """[S, E] f32 -> [ST, P(e), EC, P(s)] bf16 with [i,p,c,s] = x[128i+s, 128c+p]."""
    a = xb.reshape(ST, P, EC, P)          # [i, s, c, p]
    a = np.ascontiguousarray(a.transpose(0, 3, 2, 1))  # [i, p, c, s]
    return a.astype(BF16)


def _pack_w(w_slice: np.ndarray, n_chunks: int) -> np.ndarray:
    """[N, E] f32 (rows = out features) -> [P, EC, N] bf16, [p,c,n] = W[n, 128c+p]."""
    wt = w_slice.T                         # [E, N]
    n = w_slice.shape[0]
    a = wt.reshape(n_chunks, P, n).transpose(1, 0, 2)  # [p, c, n]
    return np.ascontiguousarray(a).astype(BF16)


def _rope_tables():
    inv_freq = 1.0 / (ROPE_BASE ** (np.arange(0, D, 2, dtype=np.float64) / D))  # [64]
    ang = np.arange(S, dtype=np.float64)[:, None] * inv_freq[None, :]           # [S, 64]
    cos = np.cos(ang).astype(F32)
    sin = np.sin(ang).astype(F32)

    def pack(t):  # [S, 64] -> [P, ST, 64] with [p, i, d] = t[128i+p, d]
        return np.ascontiguousarray(t.reshape(ST, P, 64).transpose(1, 0, 2))

    return pack(cos), pack(sin)


def _prep_inputs(q, k, v, Wq, Wk, Wv, Wd):
    cos_p, sin_p = _rope_tables()
    mask = np.triu(np.full((P, P), MASK_NEG, dtype=F32), k=1)  # 0 on/below diag
    ident = np.eye(P, dtype=F32).astype(BF16)

    xt = {}
    for b in range(NB):
        xt[("q", b)] = _pack_xt(np.asarray(q[b], dtype=F32))
        xt[("k", b)] = _pack_xt(np.asarray(k[b], dtype=F32))
        xt[("v", b)] = _pack_xt(np.asarray(v[b], dtype=F32))

    wmaps = []
    for g in range(NG):
        wq_s = _pack_w(np.asarray(Wq[g * NQ:(g + 1) * NQ], dtype=F32), EC)
        wk_s = _pack_w(np.asarray(Wk[g * NKV:(g + 1) * NKV], dtype=F32), EC)
        wv_s = _pack_w(np.asarray(Wv[g * NKV:(g + 1) * NKV], dtype=F32), EC)
        wkv_s = np.concatenate([wk_s, wv_s], axis=2)  # [P, EC, 2*NKV]
        # Wd columns slice: rows of Wd^T
        wd_s = np.asarray(Wd[:, g * NQ:(g + 1) * NQ], dtype=F32)  # [E, NQ]
        wd_t = wd_s.T  # [NQ, E] = ctx-chunk-major
        wd_p = np.ascontiguousarray(
            wd_t.reshape(CC, P, E).transpose(1, 0, 2)).astype(BF16)
        wmaps.append({"wqt": wq_s, "wkvt": wkv_s, "wdt": wd_p})

    in_maps = []
    for c in range(NCORES):
        b, g = c // NG, c % NG
        m = {
            "xtq": xt[("q", b)],
            "xtk": xt[("k", b)],
            "xtv": xt[("v", b)],
            "cosh": cos_p,
            "sinh": sin_p,
            "maskt": mask,
            "ident": ident,
        }
        m.update(wmaps[g])
        in_maps.append(m)
    return in_maps


def run(inputs: dict, trace: bool = False):
    nc = _get_module()
    in_maps = _prep_inputs(**inputs)
    res = run_bass_kernel_spmd(nc, in_maps, list(range(NCORES)), trace=trace)
    out = np.zeros((B, S, E), dtype=F32)
    for c in range(NCORES):
        out[c // NG] += res.results[c]["out"]
    return out, res


def kernel(q, k, v, Wq, Wk, Wv, Wd):
    out, _ = run(dict(q=q, k=k, v=v, Wq=Wq, Wk=Wk, Wv=Wv, Wd=Wd))
    return out



_NULL_CACHE = None


def _build_null_module():
    """Same external interface as the real module, near-zero work. Used to
    subtract the per-call PJRT/copy overhead when timing."""
    global _NULL_CACHE
    if _NULL_CACHE is not None:
        return _NULL_CACHE
    nc = bacc.Bacc()
    xtq = nc.declare_dram_parameter("xtq", [ST, P, EC, P], bf16, isOutput=False)
    xtk = nc.declare_dram_parameter("xtk", [ST, P, EC, P], bf16, isOutput=False)
    xtv = nc.declare_dram_parameter("xtv", [ST, P, EC, P], bf16, isOutput=False)
    wqt = nc.declare_dram_parameter("wqt", [P, EC, NQ], bf16, isOutput=False)
    wkvt = nc.declare_dram_parameter("wkvt", [P, EC, 2 * NKV], bf16, isOutput=False)
    wdt = nc.declare_dram_parameter("wdt", [P, CC, E], bf16, isOutput=False)
    cosh = nc.declare_dram_parameter("cosh", [P, ST, 64], fp32, isOutput=False)
    sinh = nc.declare_dram_parameter("sinh", [P, ST, 64], fp32, isOutput=False)
    maskt = nc.declare_dram_parameter("maskt", [P, P], fp32, isOutput=False)
    ident = nc.declare_dram_parameter("ident", [P, P], bf16, isOutput=False)
    out = nc.declare_dram_parameter("out", [S, E], fp32, isOutput=True)
    with tile.TileContext(nc) as tc:
        with tc.tile_pool(name="np0", bufs=1) as pool:
            t = pool.tile([P, P], fp32, tag="t")
            nc.gpsimd.dma_start(out=t, in_=maskt[:])
            nc.gpsimd.dma_start(out=out[0:P, 0:P], in_=t)
    if not nc.is_finalized():
        nc.finalize()
    _NULL_CACHE = nc
    return nc


def _get_null_module():
    return _build_null_module()

def bench(inputs: dict, iters: int = 8):
    """Measure per-execution HW time by chaining NEFF executions inside one
    jit (device-resident inputs) and taking the marginal cost per extra exec."""
    import time

    import jax
    from jax.experimental.shard_map import shard_map
    from jax.sharding import Mesh, NamedSharding, PartitionSpec

    from concourse import bass2jax, mybir as mb

    nc = _get_module()
    in_maps = _prep_inputs(**inputs)
    bass2jax.install_neuronx_cc_hook()

    in_names, out_names, out_avals = [], [], []
    partition_name = (
        nc.partition_id_tensor.name if nc.partition_id_tensor else None
    )
    for alloc in nc.m.functions[0].allocations:
        if not isinstance(alloc, mb.MemoryLocationSet):
            continue
        name = alloc.memorylocations[0].name
        if alloc.kind == "ExternalInput":
            if name != partition_name:
                in_names.append(name)
        elif alloc.kind == "ExternalOutput":
            out_names.append(name)
            out_avals.append(
                jax.core.ShapedArray(tuple(alloc.tensor_shape),
                                     mb.dt.np(alloc.dtype)))
    n_params = len(in_names)
    all_names = in_names + out_names
    if partition_name is not None:
        all_names = all_names + [partition_name]

    devices = jax.devices()[:NCORES]
    mesh = Mesh(np.asarray(devices), ("core",))
    spec = PartitionSpec("core")
    nin = n_params + len(out_names)

    concat_in = [
        np.concatenate([np.asarray(m[name]) for m in in_maps], axis=0)
        for name in in_names
    ]
    concat_zeros = [
        np.zeros((NCORES * a.shape[0], *a.shape[1:]), a.dtype) for a in out_avals
    ]
    sh = NamedSharding(mesh, spec)
    dev_in = [jax.device_put(a, sh) for a in concat_in + concat_zeros]

    def time_module(module):
        def body1(*flat):
            args, zouts = flat[:n_params], list(flat[n_params:])
            extra = (
                (bass2jax.partition_id_tensor(),) if partition_name is not None else ()
            )
            outs = bass2jax._bass_exec_p.bind(
                *args, *zouts, *extra,
                out_avals=tuple(out_avals),
                in_names=tuple(all_names),
                out_names=tuple(out_names),
                lowering_input_output_aliases=(),
                sim_require_finite=True,
                sim_require_nnan=True,
                nc=module,
            )
            return tuple(outs)

        f = jax.jit(
            shard_map(body1, mesh=mesh, in_specs=(spec,) * nin,
                      out_specs=(spec,) * len(out_names), check_rep=False),
            keep_unused=True,
        )
        r = f(*dev_in)
        jax.block_until_ready(r)
        best = float("inf")
        for _ in range(iters):
            t0 = time.perf_counter()
            r = f(*dev_in)
            jax.block_until_ready(r)
            best = min(best, time.perf_counter() - t0)
        return best

    t_full = time_module(nc)
    t_null = time_module(_get_null_module())
    print(f"  full: {t_full * 1e3:.3f} ms   null: {t_null * 1e3:.3f} ms")
    return (t_full - t_null) * 1e9
